# revision 25
# baseline (speedup 1.0000x reference)
"""EnsembleRBF TRN2 kernel: out[m,n,d] = sum_c exp(-||x_n - c_c||^2) * sigma^2 * w[m,c,d].

Data-parallel along N across 8 cores (12800 rows/core, n = p*100 + b).
Per-core pipeline (engines balanced so ACT-exp is the pacing item):
  1. prep (DVE pointwise + GPSIMD memsets): fp16 hi/lo split of x and x2 in
     natural layout; assemble aug [128, (t, 4, 32)] where each 128-col group
     holds 3 blocks' 32-comp aug rows ([xh,xh,xl, yh,yh,yl, x2h,x2l, 1,1, 0..])
     plus a 32-col pad slot.
  2. batched DMA-xbar transposes (SBUF->SBUF, 3 instructions) -> rhs_t with
     block b's comps at partitions 32*(b%3)+k, cols 128*(b//3)+p.
  3. MM1 (PE): d2[c, n] via K=32 row-tiled matmuls (tile_position=(32g,0));
     chunk = 6 blocks -> one [128,1536] fp32 PSUM tile; col layout
     g*512 + ((b-6k)//3)*256 + cc*128 keeps one row-group per PSUM bank
     (mixed row-groups in a bank hang the PE).
  4. exp split: ACT does cols [0, sa) exactly (Exp table); DVE does the tail
     dv cols with a two-phase-average Schraudolph (~+-0.75% rel):
       p  = rint(-A*d2 + B3) as u16, saturating      [per chunk, 1x from PSUM]
       q  = p + 512                                  [per 2 chunks, 4x]
       q2 = bitcast_f16(q) * rsqrt(2)                [per 2 chunks, 4x]
       v  = bitcast_f16(p) + q2                      [per 2 chunks, 2x]
     B3 = 15*1024 - 1024 + csh so v ~= exp(-d2).
  5. MM2 (PE, lagging one chunk): po[n, 16b+j] += rbf_block.T @ wr
     (j = 2m+d), accumulated in per-32-block PSUM slabs [128, 512].
  6. slab drain (ACT copy, (m,b,d)-major) -> stage -> one 4D-AP output DMA
     per slab.
"""
import numpy as np

import concourse.bass as bass
import concourse.tile as tile
from concourse import bacc, mybir
from concourse.bass_utils import run_bass_kernel_spmd

N, C, D, M = 100000, 256, 2, 5
SIGMA2 = 0.0625
NCORES = 8
NCP = 12544
NBLK = 98
NT = 33              # transposed 128-col groups (3 blocks + pad each)
CHB = 6              # blocks per chunk
NCHUNK = 17          # 16 x 6 + 1 x 4
f32 = mybir.dt.float32
f16 = mybir.dt.float16
u16 = mybir.dt.uint16

SCH_A = 1024.0 / float(np.log(2.0))     # 1477.32
SCH_B = 15 * 1024 - 1024 - 55.0         # 14281.0
RSQRT2 = 0.7071067811865476
DV = 448                                 # DVE exp cols per 1536-col chunk
DV_LAST = 320                            # for the last (4-block) chunk

_CACHE = {}


def _build():
    nc = bacc.Bacc("TRN2", target_bir_lowering=False, debug=False, num_devices=NCORES)
    x_ap = nc.dram_tensor("x", [NCP, 2], f32, kind="ExternalInput").ap()
    augc_ap = nc.dram_tensor("augc", [96, 256], f16, kind="ExternalInput").ap()
    wr_ap = nc.dram_tensor("wr", [128, 32], f16, kind="ExternalInput").ap()
    out_ap = nc.dram_tensor("out", [M, NCP, 2], f32, kind="ExternalOutput").ap()

    Exp = mybir.ActivationFunctionType.Exp
    MULT = mybir.AluOpType.mult
    ADD = mybir.AluOpType.add

    with tile.TileContext(nc) as tc:
        with (
            tc.tile_pool(name="consts", bufs=1) as consts,
            tc.tile_pool(name="d2p", bufs=2, space="PSUM") as d2_pool,
            tc.tile_pool(name="pop", bufs=2, space="PSUM") as po_pool,
        ):
            augc = consts.tile([96, 256], f16)
            wr = consts.tile([128, 32], f16)
            rbf = consts.tile([128, 26112], f16)
            aug = consts.tile([128, NT * 128], f16)
            rhs_t = consts.tile([128, NT * 128], f16)
            stage = consts.tile([128, M * NBLK * 2], f32)
            spa = consts.tile([128, NCHUNK * DV], u16)
            sqa = consts.tile([128, NCHUNK * DV], u16)
            q2a = consts.tile([128, NCHUNK * DV], f16)

            xs = consts.tile([128, 198], f32)
            sq = consts.tile([128, 198], f32)
            x2 = consts.tile([128, 99], f32)
            xh16 = consts.tile([128, 198], f16)
            xl16 = consts.tile([128, 198], f16)
            x2h16 = consts.tile([128, 99], f16)
            x2l16 = consts.tile([128, 99], f16)

            x_v = x_ap.rearrange("(p j) d -> p (j d)", p=128)
            augv = aug[:].rearrange("p (t q k) -> p t q k", q=4, k=32)
            sqv = sq[:].rearrange("p (b d) -> p b d", d=2)
            xh16g = xh16[:].rearrange("p (t g d) -> p t g d", g=3, d=2)
            xl16g = xl16[:].rearrange("p (t g d) -> p t g d", g=3, d=2)
            x2h16g = x2h16[:].rearrange("p (t g) -> p t g", g=3)
            x2l16g = x2l16[:].rearrange("p (t g) -> p t g", g=3)
            rhs_tv = rhs_t[:].rearrange("p (t a) -> p t a", a=128)

            nc.sync.dma_start(xs[:, 0:196], x_v[:])
            nc.sync.dma_start(augc[:], augc_ap[:])
            nc.sync.dma_start(wr[:], wr_ap[:])

            # pointwise prep, one pass (xs cols 200:204 are junk pad; the
            # resulting pad blocks 100/101 are never consumed by MM1)
            nc.vector.memset(xs[:, 196:198], 0.0)
            nc.vector.tensor_mul(sq[:], xs[:], xs[:])
            nc.vector.tensor_add(x2[:], sqv[:, :, 0], sqv[:, :, 1])
            nc.vector.tensor_copy(xh16[:], xs[:])
            nc.vector.tensor_sub(xl16[:], xs[:], xh16[:])
            nc.vector.tensor_copy(x2h16[:], x2[:])
            nc.vector.tensor_sub(x2l16[:], x2[:], x2h16[:])

            # aug assembly + batched transposes
            for bi, (t0, t1) in enumerate(((0, 1), (1, 3), (3, 7), (7, 14), (14, 23), (23, NT))):
                tr = slice(t0, t1)
                sh2 = (128, t1 - t0, 3, 2)
                nc.gpsimd.memset(aug[:, 128 * t0 : 128 * t1], 0.0)
                eng = nc.vector
                eng.tensor_copy(
                    augv[:, tr, 0:3, 0:2],
                    xh16g[:, tr, :, 0].unsqueeze(3).broadcast_to(sh2),
                )
                eng.tensor_copy(augv[:, tr, 0:3, 2], xl16g[:, tr, :, 0])
                eng.tensor_copy(
                    augv[:, tr, 0:3, 3:5],
                    xh16g[:, tr, :, 1].unsqueeze(3).broadcast_to(sh2),
                )
                eng.tensor_copy(augv[:, tr, 0:3, 5], xl16g[:, tr, :, 1])
                eng.tensor_copy(augv[:, tr, 0:3, 6], x2h16g[:, tr, :])
                eng.tensor_copy(augv[:, tr, 0:3, 7], x2l16g[:, tr, :])
                eng.memset(augv[:, tr, 0:3, 8:10], 1.0)
                nc.sync.dma_start_transpose(
                    rhs_tv[:, tr, :], aug[:, 128 * t0 : 128 * t1]
                )

            # ---- main chunk loop (MM2 lags one chunk) ----
            po_tiles = {}

            def dv_of(k):
                return DV if k < NCHUNK - 1 else DV_LAST

            def emit_front(k):
                bs = list(range(CHB * k, min(CHB * k + CHB, NBLK)))
                rb = 1536 * k
                dv = dv_of(k)
                sa = 1536 - dv

                d2 = d2_pool.tile([128, 1536], f32, tag="d2", name=f"d2_{k}")
                for cc in range(2):
                    for b in bs:
                        g = b % 3
                        t = b // 3
                        cb = g * 512 + ((b - CHB * k) // 3) * 256 + cc * 128
                        nc.tensor.matmul(
                            d2[:, cb : cb + 128],
                            augc[32 * g : 32 * g + 32, cc * 128 : (cc + 1) * 128],
                            rhs_t[32 * g : 32 * g + 32, 128 * t : 128 * t + 128],
                            start=True,
                            stop=True,
                            tile_position=(32 * g, 0),
                        )

                nc.scalar.activation(
                    rbf[:, rb : rb + sa], d2[:, 0:sa], Exp, scale=-1.0
                )
                nc.vector.tensor_scalar(
                    spa[:, DV * k : DV * k + dv], d2[:, sa:1536],
                    -SCH_A, SCH_B, MULT, ADD,
                )

            def emit_sch(ks):
                # ops 2-4 for a group of chunks: contiguous over spa slices
                c0, c1 = DV * ks[0], DV * ks[-1] + dv_of(ks[-1])
                nc.vector.tensor_scalar_add(sqa[:, c0:c1], spa[:, c0:c1], 512)
                nc.vector.tensor_scalar_mul(
                    q2a[:, c0:c1], sqa[:, c0:c1].bitcast(f16), RSQRT2
                )
                if len(ks) == 1:
                    k = ks[0]
                    out = rbf[:, 1536 * k + 1536 - dv_of(k) : 1536 * (k + 1)]
                else:
                    out = (
                        rbf[:]
                        .rearrange("p (k a) -> p k a", a=1536)[
                            :, ks[0] : ks[-1] + 1, 1536 - DV : 1536
                        ]
                    )
                nc.vector.tensor_add(
                    out, spa[:, c0:c1].bitcast(f16), q2a[:, c0:c1]
                )

            def emit_back(k):
                bs = list(range(CHB * k, min(CHB * k + CHB, NBLK)))
                rb = 1536 * k
                for b in bs:
                    sb = b // 32
                    if sb not in po_tiles:
                        po_t = po_pool.tile([128, 512], f32, tag="po", name=f"po_{sb}")
                        po_tiles[sb] = po_t
                    po = po_tiles[sb]
                    col = 16 * (b % 32)
                    cb0 = (b % 3) * 512 + ((b - CHB * k) // 3) * 256
                    for cc in range(2):
                        nc.tensor.matmul(
                            po[:, col : col + 16],
                            rbf[:, rb + cb0 + cc * 128 : rb + cb0 + cc * 128 + 128],
                            wr[:, 16 * cc : 16 * cc + 16],
                            start=(cc == 0),
                            stop=(cc == 1),
                        )
                return [sb for sb, lastb in ((0, 31), (1, 63), (2, 95), (3, 97))
                        if lastb in bs]

            def emit_drain(sbs):
                for sb in sbs:
                    nbl = 32 if sb < 3 else NBLK - 96
                    src = (
                        po_tiles[sb][:, 0 : 16 * nbl]
                        .rearrange("p (b j) -> p b j", j=16)[:, :, 0:10]
                        .rearrange("p b (m d) -> p m b d", d=2)
                    )
                    dst = stage[:].rearrange("p (m b d) -> p m b d", m=M, d=2)[
                        :, :, 32 * sb : 32 * sb + nbl, :
                    ]
                    nc.scalar.copy(dst, src)
                    dmadst = out_ap.rearrange("m (p b) d -> p m b d", p=128)[
                        :, :, 32 * sb : 32 * sb + nbl, :
                    ]
                    dmasrc = stage[:].rearrange("p (m b d) -> p m b d", m=M, d=2)[
                        :, :, 32 * sb : 32 * sb + nbl, :
                    ]
                    nc.sync.dma_start(dmadst, dmasrc)

            pend_mm2 = []
            pend_sch = []
            pend_drain = []
            for k in range(NCHUNK):
                emit_front(k)
                pend_sch.append(k)
                if len(pend_sch) == 2 or k == NCHUNK - 1:
                    if k == NCHUNK - 1 and len(pend_sch) == 2:
                        emit_sch(pend_sch[:1])
                        emit_sch(pend_sch[1:])
                    else:
                        emit_sch(pend_sch)
                    pend_sch = []
                pend_mm2.append(k)
                if len(pend_mm2) > 2:
                    pend_drain.append((2, emit_back(pend_mm2.pop(0))))
                pend_drain = [(d - 1, sbs) for d, sbs in pend_drain]
                for d, sbs in pend_drain:
                    if d <= 0 and sbs:
                        emit_drain(sbs)
                pend_drain = [(d, sbs) for d, sbs in pend_drain if d > 0]
            for k in pend_mm2:
                pend_drain.append((0, emit_back(k)))
            for d, sbs in pend_drain:
                if sbs:
                    emit_drain(sbs)

    nc.compile()
    return nc


def _host_prep(x, centers, weights):
    x = np.ascontiguousarray(np.asarray(x, dtype=np.float32))
    centers = np.asarray(centers, dtype=np.float32)
    weights = np.asarray(weights, dtype=np.float32)

    xp = np.zeros((NCORES * NCP, 2), np.float32)
    xp[:N] = x
    xp = xp.reshape(NCORES, NCP, 2)

    ch = centers.astype(np.float16)
    cl = (centers - ch.astype(np.float32)).astype(np.float16)
    c2 = np.sum(centers * centers, axis=1, dtype=np.float32)
    c2h = c2.astype(np.float16)
    c2l = (c2 - c2h.astype(np.float32)).astype(np.float16)
    ones = np.ones(C, np.float16)

    aug1 = np.zeros((32, 256), np.float16)
    aug1[0] = -2 * ch[:, 0]
    aug1[1] = -2 * cl[:, 0]
    aug1[2] = -2 * ch[:, 0]
    aug1[3] = -2 * ch[:, 1]
    aug1[4] = -2 * cl[:, 1]
    aug1[5] = -2 * ch[:, 1]
    aug1[6] = ones
    aug1[7] = ones
    aug1[8] = c2h
    aug1[9] = c2l
    augc = np.tile(aug1, (3, 1))  # 3 row-groups at bases 0/32/64

    wmd = (weights * SIGMA2).transpose(1, 0, 2).reshape(C, 10).astype(np.float16)
    wr = np.zeros((128, 32), np.float16)
    wr[:, 0:10] = wmd[:128]
    wr[:, 16:26] = wmd[128:]
    return xp, augc, wr


def kernel(x, centers, weights):
    if "nc" not in _CACHE:
        _CACHE["nc"] = _build()
    nc = _CACHE["nc"]
    xp, augc, wr = _host_prep(x, centers, weights)
    in_maps = [{"x": xp[i], "augc": augc, "wr": wr} for i in range(NCORES)]
    res = run_bass_kernel_spmd(nc, in_maps, list(range(NCORES)))
    outs = np.concatenate([res.results[i]["out"] for i in range(NCORES)], axis=1)
    return np.ascontiguousarray(outs[:, :N, :])


# revision 26
# speedup vs baseline: 1.0411x; 1.0411x over previous
"""EnsembleRBF TRN2 kernel: out[m,n,d] = sum_c exp(-||x_n - c_c||^2) * sigma^2 * w[m,c,d].

Data-parallel along N across 8 cores (12800 rows/core, n = p*100 + b).
Per-core pipeline (engines balanced so ACT-exp is the pacing item):
  1. prep (DVE pointwise + GPSIMD memsets): fp16 hi/lo split of x and x2 in
     natural layout; assemble aug [128, (t, 4, 32)] where each 128-col group
     holds 3 blocks' 32-comp aug rows ([xh,xh,xl, yh,yh,yl, x2h,x2l, 1,1, 0..])
     plus a 32-col pad slot.
  2. batched DMA-xbar transposes (SBUF->SBUF, 3 instructions) -> rhs_t with
     block b's comps at partitions 32*(b%3)+k, cols 128*(b//3)+p.
  3. MM1 (PE): d2[c, n] via K=32 row-tiled matmuls (tile_position=(32g,0));
     chunk = 6 blocks -> one [128,1536] fp32 PSUM tile; col layout
     g*512 + ((b-6k)//3)*256 + cc*128 keeps one row-group per PSUM bank
     (mixed row-groups in a bank hang the PE).
  4. exp split: ACT does cols [0, sa) exactly (Exp table); DVE does the tail
     dv cols with a two-phase-average Schraudolph (~+-0.75% rel):
       p  = rint(-A*d2 + B3) as u16, saturating      [per chunk, 1x from PSUM]
       q  = p + 512                                  [per 2 chunks, 4x]
       q2 = bitcast_f16(q) * rsqrt(2)                [per 2 chunks, 4x]
       v  = bitcast_f16(p) + q2                      [per 2 chunks, 2x]
     B3 = 15*1024 - 1024 + csh so v ~= exp(-d2).
  5. MM2 (PE, lagging one chunk): po[n, 16b+j] += rbf_block.T @ wr
     (j = 2m+d), accumulated in per-32-block PSUM slabs [128, 512].
  6. slab drain (ACT copy, (m,b,d)-major) -> stage -> one 4D-AP output DMA
     per slab.
"""
import numpy as np

import concourse.bass as bass
import concourse.tile as tile
from concourse import bacc, mybir
from concourse.bass_utils import run_bass_kernel_spmd

N, C, D, M = 100000, 256, 2, 5
SIGMA2 = 0.0625
NCORES = 8
NCP = 12544
NBLK = 98
NT = 33              # transposed 128-col groups (3 blocks + pad each)
CHB = 6              # blocks per chunk
NCHUNK = 17          # 16 x 6 + 1 x 4
f32 = mybir.dt.float32
f16 = mybir.dt.float16
u16 = mybir.dt.uint16

SCH_A = 1024.0 / float(np.log(2.0))     # 1477.32
SCH_B = 15 * 1024 - 1024 - 55.0         # 14281.0
RSQRT2 = 0.7071067811865476
DV = 448                                 # DVE exp cols per 1536-col chunk
DV_LAST = 320                            # for the last (4-block) chunk

_CACHE = {}


def _build():
    nc = bacc.Bacc("TRN2", target_bir_lowering=False, debug=False, num_devices=NCORES)
    x_ap = nc.dram_tensor("x", [NCP, 2], f32, kind="ExternalInput").ap()
    augc_ap = nc.dram_tensor("augc", [96, 256], f16, kind="ExternalInput").ap()
    wr_ap = nc.dram_tensor("wr", [128, 32], f16, kind="ExternalInput").ap()
    out_ap = nc.dram_tensor("out", [M, NCP, 2], f32, kind="ExternalOutput").ap()

    Exp = mybir.ActivationFunctionType.Exp
    MULT = mybir.AluOpType.mult
    ADD = mybir.AluOpType.add

    with tile.TileContext(nc) as tc:
        with (
            tc.tile_pool(name="consts", bufs=1) as consts,
            tc.tile_pool(name="d2p", bufs=2, space="PSUM") as d2_pool,
            tc.tile_pool(name="pop", bufs=2, space="PSUM") as po_pool,
        ):
            augc = consts.tile([96, 256], f16)
            wr = consts.tile([128, 32], f16)
            rbf = consts.tile([128, 26112], f16)
            aug = consts.tile([128, NT * 128], f16)
            rhs_t = consts.tile([128, NT * 128], f16)
            stage = consts.tile([128, M * NBLK * 2], f32)
            spa = consts.tile([128, NCHUNK * DV], u16)
            sqa = consts.tile([128, NCHUNK * DV], u16)
            q2a = consts.tile([128, NCHUNK * DV], f16)

            xs = consts.tile([128, 198], f32)
            sq = consts.tile([128, 198], f32)
            x2 = consts.tile([128, 99], f32)
            xh16 = consts.tile([128, 198], f16)
            xl16 = consts.tile([128, 198], f16)
            x2h16 = consts.tile([128, 99], f16)
            x2l16 = consts.tile([128, 99], f16)

            x_v = x_ap.rearrange("(p j) d -> p (j d)", p=128)
            augv = aug[:].rearrange("p (t q k) -> p t q k", q=4, k=32)
            sqv = sq[:].rearrange("p (b d) -> p b d", d=2)
            xh16g = xh16[:].rearrange("p (t g d) -> p t g d", g=3, d=2)
            xl16g = xl16[:].rearrange("p (t g d) -> p t g d", g=3, d=2)
            x2h16g = x2h16[:].rearrange("p (t g) -> p t g", g=3)
            x2l16g = x2l16[:].rearrange("p (t g) -> p t g", g=3)
            rhs_tv = rhs_t[:].rearrange("p (t a) -> p t a", a=128)

            nc.sync.dma_start(xs[:, 0:196], x_v[:])
            nc.sync.dma_start(augc[:], augc_ap[:])
            nc.sync.dma_start(wr[:], wr_ap[:])

            # pointwise prep, one pass (xs cols 200:204 are junk pad; the
            # resulting pad blocks 100/101 are never consumed by MM1)
            nc.vector.memset(xs[:, 196:198], 0.0)
            nc.vector.tensor_mul(sq[:], xs[:], xs[:])
            nc.vector.tensor_add(x2[:], sqv[:, :, 0], sqv[:, :, 1])
            nc.vector.tensor_copy(xh16[:], xs[:])
            nc.vector.tensor_sub(xl16[:], xs[:], xh16[:])
            nc.vector.tensor_copy(x2h16[:], x2[:])
            nc.vector.tensor_sub(x2l16[:], x2[:], x2h16[:])

            # aug assembly + batched transposes
            for bi, (t0, t1) in enumerate(((0, 2), (2, 5), (5, 12), (12, 22), (22, NT))):
                tr = slice(t0, t1)
                sh2 = (128, t1 - t0, 3, 2)
                nc.gpsimd.memset(aug[:, 128 * t0 : 128 * t1], 0.0)
                eng = nc.vector
                eng.tensor_copy(
                    augv[:, tr, 0:3, 0:2],
                    xh16g[:, tr, :, 0].unsqueeze(3).broadcast_to(sh2),
                )
                eng.tensor_copy(augv[:, tr, 0:3, 2], xl16g[:, tr, :, 0])
                eng.tensor_copy(
                    augv[:, tr, 0:3, 3:5],
                    xh16g[:, tr, :, 1].unsqueeze(3).broadcast_to(sh2),
                )
                eng.tensor_copy(augv[:, tr, 0:3, 5], xl16g[:, tr, :, 1])
                eng.tensor_copy(augv[:, tr, 0:3, 6], x2h16g[:, tr, :])
                eng.tensor_copy(augv[:, tr, 0:3, 7], x2l16g[:, tr, :])
                eng.memset(augv[:, tr, 0:3, 8:10], 1.0)
                nc.sync.dma_start_transpose(
                    rhs_tv[:, tr, :], aug[:, 128 * t0 : 128 * t1]
                )

            # ---- main chunk loop (MM2 lags one chunk) ----
            po_tiles = {}

            def dv_of(k):
                return DV if k < NCHUNK - 1 else DV_LAST

            def emit_front(k):
                bs = list(range(CHB * k, min(CHB * k + CHB, NBLK)))
                rb = 1536 * k
                dv = dv_of(k)
                sa = 1536 - dv

                d2 = d2_pool.tile([128, 1536], f32, tag="d2", name=f"d2_{k}")
                for cc in range(2):
                    for b in bs:
                        g = b % 3
                        t = b // 3
                        cb = g * 512 + ((b - CHB * k) // 3) * 256 + cc * 128
                        nc.tensor.matmul(
                            d2[:, cb : cb + 128],
                            augc[32 * g : 32 * g + 32, cc * 128 : (cc + 1) * 128],
                            rhs_t[32 * g : 32 * g + 32, 128 * t : 128 * t + 128],
                            start=True,
                            stop=True,
                            tile_position=(32 * g, 0),
                        )

                nc.scalar.activation(
                    rbf[:, rb : rb + sa], d2[:, 0:sa], Exp, scale=-1.0
                )
                nc.vector.tensor_scalar(
                    spa[:, DV * k : DV * k + dv], d2[:, sa:1536],
                    -SCH_A, SCH_B, MULT, ADD,
                )

            def emit_sch(ks):
                # ops 2-4 for a group of chunks: contiguous over spa slices
                c0, c1 = DV * ks[0], DV * ks[-1] + dv_of(ks[-1])
                nc.vector.tensor_scalar_add(sqa[:, c0:c1], spa[:, c0:c1], 512)
                nc.vector.tensor_scalar_mul(
                    q2a[:, c0:c1], sqa[:, c0:c1].bitcast(f16), RSQRT2
                )
                if len(ks) == 1:
                    k = ks[0]
                    out = rbf[:, 1536 * k + 1536 - dv_of(k) : 1536 * (k + 1)]
                else:
                    out = (
                        rbf[:]
                        .rearrange("p (k a) -> p k a", a=1536)[
                            :, ks[0] : ks[-1] + 1, 1536 - DV : 1536
                        ]
                    )
                nc.vector.tensor_add(
                    out, spa[:, c0:c1].bitcast(f16), q2a[:, c0:c1]
                )

            def emit_back(k):
                bs = list(range(CHB * k, min(CHB * k + CHB, NBLK)))
                rb = 1536 * k
                for b in bs:
                    sb = b // 32
                    if sb not in po_tiles:
                        po_t = po_pool.tile([128, 512], f32, tag="po", name=f"po_{sb}")
                        po_tiles[sb] = po_t
                    po = po_tiles[sb]
                    col = 16 * (b % 32)
                    cb0 = (b % 3) * 512 + ((b - CHB * k) // 3) * 256
                    for cc in range(2):
                        nc.tensor.matmul(
                            po[:, col : col + 16],
                            rbf[:, rb + cb0 + cc * 128 : rb + cb0 + cc * 128 + 128],
                            wr[:, 16 * cc : 16 * cc + 16],
                            start=(cc == 0),
                            stop=(cc == 1),
                        )
                return [sb for sb, lastb in ((0, 31), (1, 63), (2, 95), (3, 97))
                        if lastb in bs]

            def emit_drain(sbs):
                for sb in sbs:
                    nbl = 32 if sb < 3 else NBLK - 96
                    src = (
                        po_tiles[sb][:, 0 : 16 * nbl]
                        .rearrange("p (b j) -> p b j", j=16)[:, :, 0:10]
                        .rearrange("p b (m d) -> p m b d", d=2)
                    )
                    dst = stage[:].rearrange("p (m b d) -> p m b d", m=M, d=2)[
                        :, :, 32 * sb : 32 * sb + nbl, :
                    ]
                    nc.scalar.copy(dst, src)
                    dmadst = out_ap.rearrange("m (p b) d -> p m b d", p=128)[
                        :, :, 32 * sb : 32 * sb + nbl, :
                    ]
                    dmasrc = stage[:].rearrange("p (m b d) -> p m b d", m=M, d=2)[
                        :, :, 32 * sb : 32 * sb + nbl, :
                    ]
                    nc.sync.dma_start(dmadst, dmasrc)

            pend_mm2 = []
            pend_sch = []
            pend_drain = []
            for k in range(NCHUNK):
                emit_front(k)
                pend_sch.append(k)
                if len(pend_sch) == 2 or k == NCHUNK - 1:
                    if k == NCHUNK - 1 and len(pend_sch) == 2:
                        emit_sch(pend_sch[:1])
                        emit_sch(pend_sch[1:])
                    else:
                        emit_sch(pend_sch)
                    pend_sch = []
                pend_mm2.append(k)
                if len(pend_mm2) > 2:
                    pend_drain.append((2, emit_back(pend_mm2.pop(0))))
                pend_drain = [(d - 1, sbs) for d, sbs in pend_drain]
                for d, sbs in pend_drain:
                    if d <= 0 and sbs:
                        emit_drain(sbs)
                pend_drain = [(d, sbs) for d, sbs in pend_drain if d > 0]
            for k in pend_mm2:
                pend_drain.append((0, emit_back(k)))
            for d, sbs in pend_drain:
                if sbs:
                    emit_drain(sbs)

    nc.compile()
    return nc


def _host_prep(x, centers, weights):
    x = np.ascontiguousarray(np.asarray(x, dtype=np.float32))
    centers = np.asarray(centers, dtype=np.float32)
    weights = np.asarray(weights, dtype=np.float32)

    xp = np.zeros((NCORES * NCP, 2), np.float32)
    xp[:N] = x
    xp = xp.reshape(NCORES, NCP, 2)

    ch = centers.astype(np.float16)
    cl = (centers - ch.astype(np.float32)).astype(np.float16)
    c2 = np.sum(centers * centers, axis=1, dtype=np.float32)
    c2h = c2.astype(np.float16)
    c2l = (c2 - c2h.astype(np.float32)).astype(np.float16)
    ones = np.ones(C, np.float16)

    aug1 = np.zeros((32, 256), np.float16)
    aug1[0] = -2 * ch[:, 0]
    aug1[1] = -2 * cl[:, 0]
    aug1[2] = -2 * ch[:, 0]
    aug1[3] = -2 * ch[:, 1]
    aug1[4] = -2 * cl[:, 1]
    aug1[5] = -2 * ch[:, 1]
    aug1[6] = ones
    aug1[7] = ones
    aug1[8] = c2h
    aug1[9] = c2l
    augc = np.tile(aug1, (3, 1))  # 3 row-groups at bases 0/32/64

    wmd = (weights * SIGMA2).transpose(1, 0, 2).reshape(C, 10).astype(np.float16)
    wr = np.zeros((128, 32), np.float16)
    wr[:, 0:10] = wmd[:128]
    wr[:, 16:26] = wmd[128:]
    return xp, augc, wr


def kernel(x, centers, weights):
    if "nc" not in _CACHE:
        _CACHE["nc"] = _build()
    nc = _CACHE["nc"]
    xp, augc, wr = _host_prep(x, centers, weights)
    in_maps = [{"x": xp[i], "augc": augc, "wr": wr} for i in range(NCORES)]
    res = run_bass_kernel_spmd(nc, in_maps, list(range(NCORES)))
    outs = np.concatenate([res.results[i]["out"] for i in range(NCORES)], axis=1)
    return np.ascontiguousarray(outs[:, :N, :])
